# revision 19
# baseline (speedup 1.0000x reference)
"""Adaptive polyphase sampling (stride 2, p=2) on 8 TRN2 NeuronCores.

For x [32, 256, 64, 64] f32: compute the 4 polyphase components
x[:, :, i::2, j::2], pick per-sample the component with the largest L2
norm (over channels+space), return it [32, 256, 32, 32].

Sharding: pure data parallel over batch — 4 samples per core, no
cross-core communication.

Layout: partition p holds the channel pair {2p, 2p+1}; each sample is
one contiguous 32 KiB run per partition.

DMA: one HWDGE ring issuing back-to-back transfers reaches ~400 GB/s
and drains FIFO, so ALL data movement lives on the sync ring: 4 input
transfers queued immediately, then the 4 output transfers (their
descriptors enqueue when each result is ready and drain after the
inputs — no bandwidth contention during the input phase).

Per-core dataflow (samples s = 0..3):
  sync   : DMA x[s] -> samp[s] (back-to-back); DMA obuf[s%3] -> out[s]
  scalar : norms k=0..3 (Square activation + accum_out); sample 3 only
           k=0,1 (k=2,3 go to vector to shorten the tail); Square
           activation table preloaded via a zero-scale dummy op
  vector : sample-3 norms k=2,3 (scalar_tensor_tensor square+accum);
           mask: reduce_max(psum) + is_equal -> mask[:, 4s:4s+4];
           select: c = V0*m0; c = (V1*m1)+c; c = (V2*m2)+c;
           obuf = (V3*m3)+c                  (scalar_tensor_tensor)
  tensor : ones[128,128] @ norms -> psum (channel reduce + broadcast)
  gpsimd : memset ones (then idle; Block(no_gpsimd_drain) skips its
           expensive end-of-kernel DGE drain)

The argmax is realized as mask_k = (norm_k == max_k norm_k) in {0,1},
then O = sum_k mask_k * V_k. Exact float ties between component norms
(sums of ~1M random squares) are probability-zero.

Synchronization: engines are pipelined, so same-engine data deps need
semaphore handshakes (writes land at DRAIN). Every compute op on
scalar/vector increments its engine chain sem (sch/vch) and waits for
all previously-emitted ops on that engine; cross-engine waits
reference chain thresholds from the static emission plan below. Every
in-flight DMA gets its own semaphore (completions across DMAs are not
ordered by the semaphore protocol).
"""

from contextlib import ExitStack

import numpy as np

import concourse.bass as bass
from concourse import mybir
from concourse.bass_utils import run_bass_kernel_spmd

F32 = mybir.dt.float32
AX = mybir.AxisListType
OP = mybir.AluOpType
ACT = mybir.ActivationFunctionType

B, C, H, W = 32, 256, 64, 64
NCORES = 8
SPC = B // NCORES          # samples per core
H2, W2 = H // 2, W // 2    # 32, 32
SP = H * W                 # 4096 spatial elems per channel
OSP = H2 * W2              # 1024

N_SAMP_BUFS = 4
N_OBUFS = 3
LAST = SPC - 1

# ---- static emission plan ------------------------------------------------
# scalar: op 0 is the table-preload dummy; 4 norms per sample, except
# sample 2 whose k=3 norm runs on vector (unblocks its mask earlier)
SC_ORDER = [("pre", 0), ("n", 0), ("n", 1), ("n", 2), ("n", LAST)]
SC_SIZES = {("pre", 0): 1, ("n", 0): 4, ("n", 1): 4, ("n", 2): 3, ("n", LAST): 4}
# vector: mk = 2 mask ops ; ch = 4 chain ops (8 for the last sample,
# which runs per channel plane so its first output DMA can overlap)
VE_ORDER = [("mk", 0), ("ch", 0), ("mk", 1), ("ch", 1), ("vn", 2),
            ("mk", 2), ("ch", 2), ("mk", 3), ("ch", 3)]
VE_SIZES = {("ch", s): (8 if s == LAST else 4) for s in range(SPC)}
VE_SIZES.update({("mk", s): 2 for s in range(SPC)})
VE_SIZES[("vn", 2)] = 1
# vch threshold after the last sample's plane-0 chain (first 4 of its 8 ops)
# gpsimd: just the ones-memset
GP_ORDER = [("ones", 0)]
GP_SIZES = {"ones": 1}


def _plan(order, sizes):
    done, start, c = {}, {}, 0
    for key in order:
        start[key] = c
        c += sizes[key] if key in sizes else sizes[key[0]]
        done[key] = c
    return done, start, c


SC_DONE, SC_START, SC_TOTAL = _plan(SC_ORDER, SC_SIZES)
VE_DONE, VE_START, VE_TOTAL = _plan(VE_ORDER, VE_SIZES)
GP_DONE, GP_START, GP_TOTAL = _plan(GP_ORDER, GP_SIZES)
VE_LAST_P0 = VE_START[("ch", LAST)] + 4


def build_nc():
    nc = bass.Bass("TRN2", target_bir_lowering=False, debug=False)
    x = nc.dram_tensor("x", [SPC, C, H, W], F32, kind="ExternalInput")
    out = nc.dram_tensor("out", [SPC, C, H2, W2], F32, kind="ExternalOutput")

    # x[s] as [128, 2, 4096]: partition p <- channel pair {2p, 2p+1}
    x_aps = [
        x.ap()[s].rearrange("(p c) h w -> p c (h w)", c=2) for s in range(SPC)
    ]
    out_aps = [
        out.ap()[s].rearrange("(p c) a b -> p c (a b)", c=2) for s in range(SPC)
    ]

    with ExitStack() as ctx:
        block = ctx.enter_context(nc.Block(no_gpsimd_drain=True))
        sem = lambda name: ctx.enter_context(nc.semaphore(name))
        sb = lambda name, shape: ctx.enter_context(nc.sbuf_tensor(name, shape, F32))
        dmains = [sem(f"dmain{i}") for i in range(SPC)]
        dmaouts = [sem(f"dmaout{i}") for i in range(SPC + 1)]
        sch, vch, gch, mm = sem("sch"), sem("vch"), sem("gch"), sem("mm")
        samps = [sb(f"samp{i}", [128, 2, SP]) for i in range(N_SAMP_BUFS)]
        obufs = [sb(f"obuf{i}", [128, 2, OSP]) for i in range(N_OBUFS)]
        cb1 = sb("cb1", [128, 2, OSP])
        cb2 = sb("cb2", [128, 2, OSP])
        sqs = sb("sqs", [128, 2, OSP])
        sqv = sb("sqv", [128, 2, OSP])
        norms = sb("norms", [128, 4 * SPC])
        mask = sb("mask", [128, 4 * SPC])
        mx = sb("mx", [128, SPC])
        ones = sb("ones", [128, 128])
        psums = [
            ctx.enter_context(nc.psum_tensor(f"ps{i}", [128, 4], F32))
            for i in range(2)
        ]

        def V(s, k):
            i, j = divmod(k, 2)
            v6 = samps[s % N_SAMP_BUFS].ap().rearrange(
                "p c (r i q j) -> p c r i q j", r=H2, i=2, q=W2, j=2
            )
            return v6[:, :, :, i, :, j]

        sq_view = lambda t: t.ap().rearrange("p c (r q) -> p c r q", r=H2)
        ncol = lambda s, k: norms.ap()[:, 4 * s + k : 4 * s + k + 1]
        mcol = lambda s, k: mask.ap()[:, 4 * s + k : 4 * s + k + 1]

        @block.sync
        def _(sync):
            for s in range(SPC):
                sync.dma_start(out=samps[s].ap(), in_=x_aps[s]).then_inc(
                    dmains[s], 16
                )
            sync.wait_ge(dmains[LAST], 16)
            for s in range(SPC - 1):
                sync.wait_ge(vch, VE_DONE[("ch", s)])
                sync.dma_start(out=out_aps[s], in_=obufs[s % N_OBUFS].ap()).then_inc(
                    dmaouts[s], 16
                )
            # last sample: plane 0 out as soon as its half-chain finishes
            ob = obufs[LAST % N_OBUFS].ap()
            sync.wait_ge(vch, VE_LAST_P0)
            sync.dma_start(out=out_aps[LAST][:, 0], in_=ob[:, 0]).then_inc(
                dmaouts[LAST], 16
            )
            sync.wait_ge(vch, VE_DONE[("ch", LAST)])
            sync.dma_start(out=out_aps[LAST][:, 1], in_=ob[:, 1]).then_inc(
                dmaouts[LAST + 1], 16
            )
            # all outputs must land before the kernel may retire
            for s in range(SPC + 1):
                sync.wait_ge(dmaouts[s], 16)

        @block.gpsimd
        def _(gpsimd):
            gcnt = [0]

            def gemit(inst):
                inst.then_inc(gch, 1)
                gcnt[0] += 1

            def gbarrier():
                if gcnt[0]:
                    gpsimd.wait_ge(gch, gcnt[0])

            gemit(gpsimd.memset(ones.ap(), 1.0))

        @block.tensor
        def _(tensor):
            tensor.wait_ge(gch, 1)
            for s in range(SPC):
                tensor.wait_ge(sch, SC_DONE[("n", s)])
                if s == 2:
                    tensor.wait_ge(vch, VE_DONE[("vn", 2)])
                if s >= 2:
                    tensor.wait_ge(vch, VE_DONE[("mk", s - 2)])
                tensor.matmul(
                    psums[s % 2].ap(),
                    ones.ap(),
                    norms.ap()[:, 4 * s : 4 * s + 4],
                    start=True,
                    stop=True,
                ).then_inc(mm, 1)

        @block.scalar
        def _(scalar):
            cnt = [0]

            def emit(inst):
                inst.then_inc(sch, 1)
                cnt[0] += 1

            def barrier():
                if cnt[0]:
                    scalar.wait_ge(sch, cnt[0])

            # preload the Square activation table before any data arrives
            # (scale=0.0 makes the engine skip reading the input)
            emit(
                scalar.activation(
                    sqs.ap()[:, 0, 0:1], sqs.ap()[:, 0, 0:1], ACT.Square, scale=0.0
                )
            )

            sqh = sq_view(sqs)
            for g, s in SC_ORDER[1:]:
                scalar.wait_ge(dmains[s], 16)
                ks = (0, 1, 2) if s == 2 else (0, 1, 2, 3)
                for k in ks:
                    barrier()
                    emit(
                        scalar.activation(
                            sqh, V(s, k), ACT.Square, accum_out=ncol(s, k)
                        )
                    )
            assert cnt[0] == SC_TOTAL

        @block.vector
        def _(vector):
            cnt = [0]

            def emit(inst):
                inst.then_inc(vch, 1)
                cnt[0] += 1

            def barrier():
                if cnt[0]:
                    vector.wait_ge(vch, cnt[0])

            def mk(s):
                vector.wait_ge(mm, s + 1)
                barrier()
                emit(
                    vector.reduce_max(
                        mx.ap()[:, s : s + 1], psums[s % 2].ap(), axis=AX.X
                    )
                )
                barrier()
                emit(
                    vector.tensor_scalar(
                        out=mask.ap()[:, 4 * s : 4 * s + 4],
                        in0=psums[s % 2].ap(),
                        scalar1=mx.ap()[:, s : s + 1],
                        scalar2=None,
                        op0=OP.is_equal,
                    )
                )

            def vn(s):
                vector.wait_ge(dmains[s], 16)
                barrier()
                emit(
                    vector.scalar_tensor_tensor(
                        out=sq_view(sqv),
                        in0=V(s, 3),
                        scalar=0.0,
                        in1=V(s, 3),
                        op0=OP.bypass,
                        op1=OP.mult,
                        accum_out=ncol(s, 3),
                    )
                )

            def ch_planes(s):
                for plane in range(2):
                    vv = lambda k: V(s, k)[:, plane]
                    c1 = cb1.ap()[:, plane]
                    c2 = cb2.ap()[:, plane]
                    barrier()
                    emit(vector.tensor_scalar_mul(c1, vv(0), mcol(s, 0)))
                    barrier()
                    emit(
                        vector.scalar_tensor_tensor(
                            out=c2, in0=vv(1), scalar=mcol(s, 1), in1=c1,
                            op0=OP.mult, op1=OP.add,
                        )
                    )
                    barrier()
                    emit(
                        vector.scalar_tensor_tensor(
                            out=c1, in0=vv(2), scalar=mcol(s, 2), in1=c2,
                            op0=OP.mult, op1=OP.add,
                        )
                    )
                    if s >= N_OBUFS and plane == 0:
                        vector.wait_ge(dmaouts[s - N_OBUFS], 16)
                    barrier()
                    emit(
                        vector.scalar_tensor_tensor(
                            out=obufs[s % N_OBUFS].ap()[:, plane], in0=vv(3),
                            scalar=mcol(s, 3), in1=c1, op0=OP.mult, op1=OP.add,
                        )
                    )

            def ch(s):
                if s == LAST:
                    ch_planes(s)
                    return
                barrier()
                emit(vector.tensor_scalar_mul(sq_view(cb1), V(s, 0), mcol(s, 0)))
                barrier()
                emit(
                    vector.scalar_tensor_tensor(
                        out=cb2.ap(), in0=V(s, 1), scalar=mcol(s, 1), in1=cb1.ap(),
                        op0=OP.mult, op1=OP.add,
                    )
                )
                barrier()
                emit(
                    vector.scalar_tensor_tensor(
                        out=cb1.ap(), in0=V(s, 2), scalar=mcol(s, 2), in1=cb2.ap(),
                        op0=OP.mult, op1=OP.add,
                    )
                )
                if s >= N_OBUFS:
                    vector.wait_ge(dmaouts[s - N_OBUFS], 16)
                barrier()
                emit(
                    vector.scalar_tensor_tensor(
                        out=obufs[s % N_OBUFS].ap(), in0=V(s, 3), scalar=mcol(s, 3),
                        in1=cb1.ap(), op0=OP.mult, op1=OP.add,
                    )
                )

            fns = {"mk": mk, "ch": ch, "vn": vn}
            for g, s in VE_ORDER:
                fns[g](s)
            assert cnt[0] == VE_TOTAL

    return nc


_NC_CACHE = None


def _get_nc():
    global _NC_CACHE
    if _NC_CACHE is None:
        _NC_CACHE = build_nc()
    return _NC_CACHE


def kernel(x) -> np.ndarray:
    x = np.asarray(x, dtype=np.float32)
    assert x.shape == (B, C, H, W), x.shape
    shards = np.split(x, NCORES, axis=0)
    in_maps = [{"x": s} for s in shards]
    res = run_bass_kernel_spmd(_get_nc(), in_maps, core_ids=list(range(NCORES)))
    return np.concatenate([r["out"] for r in res.results], axis=0)


# revision 20
# speedup vs baseline: 1.0907x; 1.0907x over previous
"""Adaptive polyphase sampling (stride 2, p=2) on 8 TRN2 NeuronCores.

For x [32, 256, 64, 64] f32: compute the 4 polyphase components
x[:, :, i::2, j::2], pick per-sample the component with the largest L2
norm (over channels+space), return it [32, 256, 32, 32].

Sharding: pure data parallel over batch — 4 samples per core, no
cross-core communication.

Layout: partition p holds the channel pair {2p, 2p+1}; each sample is
one contiguous 32 KiB run per partition.

DMA: one HWDGE ring issuing back-to-back transfers reaches ~400 GB/s
and drains FIFO, so ALL data movement lives on the sync ring: 4 input
transfers queued immediately, then the 4 output transfers (their
descriptors enqueue when each result is ready and drain after the
inputs — no bandwidth contention during the input phase).

Per-core dataflow (samples s = 0..3):
  sync   : DMA x[s] -> samp[s] (back-to-back); DMA obuf[s%3] -> out[s]
  scalar : norms k=0..3 (Square activation + accum_out); sample 3 only
           k=0,1 (k=2,3 go to vector to shorten the tail); Square
           activation table preloaded via a zero-scale dummy op
  vector : sample-3 norms k=2,3 (scalar_tensor_tensor square+accum);
           mask: reduce_max(psum) + is_equal -> mask[:, 4s:4s+4];
           select: c = V0*m0; c = (V1*m1)+c; c = (V2*m2)+c;
           obuf = (V3*m3)+c                  (scalar_tensor_tensor)
  tensor : ones[128,128] @ norms -> psum (channel reduce + broadcast)
  gpsimd : memset ones (then idle; Block(no_gpsimd_drain) skips its
           expensive end-of-kernel DGE drain)

The argmax is realized as mask_k = (norm_k == max_k norm_k) in {0,1},
then O = sum_k mask_k * V_k. Exact float ties between component norms
(sums of ~1M random squares) are probability-zero.

Synchronization: engines are pipelined, so same-engine data deps need
semaphore handshakes (writes land at DRAIN). Every compute op on
scalar/vector increments its engine chain sem (sch/vch) and waits for
all previously-emitted ops on that engine; cross-engine waits
reference chain thresholds from the static emission plan below. Every
in-flight DMA gets its own semaphore (completions across DMAs are not
ordered by the semaphore protocol).
"""

from contextlib import ExitStack

import numpy as np

import concourse.bass as bass
from concourse import mybir
from concourse.bass_utils import run_bass_kernel_spmd

F32 = mybir.dt.float32
AX = mybir.AxisListType
OP = mybir.AluOpType
ACT = mybir.ActivationFunctionType

B, C, H, W = 32, 256, 64, 64
NCORES = 8
SPC = B // NCORES          # samples per core
H2, W2 = H // 2, W // 2    # 32, 32
SP = H * W                 # 4096 spatial elems per channel
OSP = H2 * W2              # 1024

N_SAMP_BUFS = 4
N_OBUFS = 3
LAST = SPC - 1

# ---- static emission plan ------------------------------------------------
# scalar: op 0 is the table-preload dummy; 4 norms per sample, except
# sample 2 whose k=3 norm runs on vector (unblocks its mask earlier)
SC_ORDER = [("pre", 0), ("n", 0), ("n", 1), ("n", 2), ("n", LAST)]
SC_SIZES = {("pre", 0): 1, ("n", 0): 4, ("n", 1): 4, ("n", 2): 4, ("n", LAST): 4}
# vector: mk = 2 mask ops ; ch = 4 chain ops (8 for the last sample,
# which runs per channel plane so its first output DMA can overlap)
VE_ORDER = [("mk", 0), ("ch", 0), ("mk", 1), ("ch", 1),
            ("mk", 2), ("ch", 2), ("mk", 3), ("ch", 3)]
VE_SIZES = {("ch", s): (8 if s == LAST else 4) for s in range(SPC)}
VE_SIZES.update({("mk", s): 2 for s in range(SPC)})

# vch threshold after the last sample's plane-0 chain (first 4 of its 8 ops)
# gpsimd: just the ones-memset
GP_ORDER = [("ones", 0)]
GP_SIZES = {"ones": 1}


def _plan(order, sizes):
    done, start, c = {}, {}, 0
    for key in order:
        start[key] = c
        c += sizes[key] if key in sizes else sizes[key[0]]
        done[key] = c
    return done, start, c


SC_DONE, SC_START, SC_TOTAL = _plan(SC_ORDER, SC_SIZES)
VE_DONE, VE_START, VE_TOTAL = _plan(VE_ORDER, VE_SIZES)
GP_DONE, GP_START, GP_TOTAL = _plan(GP_ORDER, GP_SIZES)
VE_LAST_P0 = VE_START[("ch", LAST)] + 4


def build_nc():
    nc = bass.Bass("TRN2", target_bir_lowering=False, debug=False)
    x = nc.dram_tensor("x", [SPC, C, H, W], F32, kind="ExternalInput")
    out = nc.dram_tensor("out", [SPC, C, H2, W2], F32, kind="ExternalOutput")

    # x[s] as [128, 2, 4096]: partition p <- channel pair {2p, 2p+1}
    x_aps = [
        x.ap()[s].rearrange("(p c) h w -> p c (h w)", c=2) for s in range(SPC)
    ]
    out_aps = [
        out.ap()[s].rearrange("(p c) a b -> p c (a b)", c=2) for s in range(SPC)
    ]

    with ExitStack() as ctx:
        block = ctx.enter_context(nc.Block(no_gpsimd_drain=True))
        sem = lambda name: ctx.enter_context(nc.semaphore(name))
        sb = lambda name, shape: ctx.enter_context(nc.sbuf_tensor(name, shape, F32))
        dmains = [sem(f"dmain{i}") for i in range(SPC)]
        dmaouts = [sem(f"dmaout{i}") for i in range(SPC + 1)]
        sch, vch, gch, mm = sem("sch"), sem("vch"), sem("gch"), sem("mm")
        samps = [sb(f"samp{i}", [128, 2, SP]) for i in range(N_SAMP_BUFS)]
        obufs = [sb(f"obuf{i}", [128, 2, OSP]) for i in range(N_OBUFS)]
        cb1 = sb("cb1", [128, 2, OSP])
        cb2 = sb("cb2", [128, 2, OSP])
        sqs = sb("sqs", [128, 2, OSP])
        sqv = sb("sqv", [128, 2, OSP])
        norms = sb("norms", [128, 4 * SPC])
        mask = sb("mask", [128, 4 * SPC])
        mx = sb("mx", [128, SPC])
        ones = sb("ones", [128, 128])
        psums = [
            ctx.enter_context(nc.psum_tensor(f"ps{i}", [128, 4], F32))
            for i in range(2)
        ]

        def V(s, k):
            i, j = divmod(k, 2)
            v6 = samps[s % N_SAMP_BUFS].ap().rearrange(
                "p c (r i q j) -> p c r i q j", r=H2, i=2, q=W2, j=2
            )
            return v6[:, :, :, i, :, j]

        sq_view = lambda t: t.ap().rearrange("p c (r q) -> p c r q", r=H2)
        ncol = lambda s, k: norms.ap()[:, 4 * s + k : 4 * s + k + 1]
        mcol = lambda s, k: mask.ap()[:, 4 * s + k : 4 * s + k + 1]

        @block.sync
        def _(sync):
            for s in range(SPC):
                sync.dma_start(out=samps[s].ap(), in_=x_aps[s]).then_inc(
                    dmains[s], 16
                )
            sync.wait_ge(dmains[LAST], 16)
            for s in range(SPC - 1):
                sync.wait_ge(vch, VE_DONE[("ch", s)])
                sync.dma_start(out=out_aps[s], in_=obufs[s % N_OBUFS].ap()).then_inc(
                    dmaouts[s], 16
                )
            # last sample: plane 0 out as soon as its half-chain finishes
            ob = obufs[LAST % N_OBUFS].ap()
            sync.wait_ge(vch, VE_LAST_P0)
            sync.dma_start(out=out_aps[LAST][:, 0], in_=ob[:, 0]).then_inc(
                dmaouts[LAST], 16
            )
            sync.wait_ge(vch, VE_DONE[("ch", LAST)])
            sync.dma_start(out=out_aps[LAST][:, 1], in_=ob[:, 1]).then_inc(
                dmaouts[LAST + 1], 16
            )
            # all outputs must land before the kernel may retire
            for s in range(SPC + 1):
                sync.wait_ge(dmaouts[s], 16)

        @block.gpsimd
        def _(gpsimd):
            gcnt = [0]

            def gemit(inst):
                inst.then_inc(gch, 1)
                gcnt[0] += 1

            def gbarrier():
                if gcnt[0]:
                    gpsimd.wait_ge(gch, gcnt[0])

            gemit(gpsimd.memset(ones.ap(), 1.0))

        @block.tensor
        def _(tensor):
            tensor.wait_ge(gch, 1)
            for s in range(SPC):
                tensor.wait_ge(sch, SC_DONE[("n", s)])
                if s >= 2:
                    tensor.wait_ge(vch, VE_DONE[("mk", s - 2)])
                tensor.matmul(
                    psums[s % 2].ap(),
                    ones.ap(),
                    norms.ap()[:, 4 * s : 4 * s + 4],
                    start=True,
                    stop=True,
                ).then_inc(mm, 1)

        @block.scalar
        def _(scalar):
            cnt = [0]

            def emit(inst):
                inst.then_inc(sch, 1)
                cnt[0] += 1

            def barrier():
                if cnt[0]:
                    scalar.wait_ge(sch, cnt[0])

            # preload the Square activation table before any data arrives
            # (scale=0.0 makes the engine skip reading the input)
            emit(
                scalar.activation(
                    sqs.ap()[:, 0, 0:1], sqs.ap()[:, 0, 0:1], ACT.Square, scale=0.0
                )
            )

            sqh = sq_view(sqs)
            for g, s in SC_ORDER[1:]:
                scalar.wait_ge(dmains[s], 16)
                for k in (0, 1, 2, 3):
                    barrier()
                    emit(
                        scalar.activation(
                            sqh, V(s, k), ACT.Square, accum_out=ncol(s, k)
                        )
                    )
            assert cnt[0] == SC_TOTAL

        @block.vector
        def _(vector):
            cnt = [0]

            def emit(inst):
                inst.then_inc(vch, 1)
                cnt[0] += 1

            def barrier():
                if cnt[0]:
                    vector.wait_ge(vch, cnt[0])

            def mk(s):
                vector.wait_ge(mm, s + 1)
                barrier()
                emit(
                    vector.reduce_max(
                        mx.ap()[:, s : s + 1], psums[s % 2].ap(), axis=AX.X
                    )
                )
                barrier()
                emit(
                    vector.tensor_scalar(
                        out=mask.ap()[:, 4 * s : 4 * s + 4],
                        in0=psums[s % 2].ap(),
                        scalar1=mx.ap()[:, s : s + 1],
                        scalar2=None,
                        op0=OP.is_equal,
                    )
                )

            def vn(s):
                vector.wait_ge(dmains[s], 16)
                barrier()
                emit(
                    vector.scalar_tensor_tensor(
                        out=sq_view(sqv),
                        in0=V(s, 3),
                        scalar=0.0,
                        in1=V(s, 3),
                        op0=OP.bypass,
                        op1=OP.mult,
                        accum_out=ncol(s, 3),
                    )
                )

            def ch_planes(s):
                for plane in range(2):
                    vv = lambda k: V(s, k)[:, plane]
                    c1 = cb1.ap()[:, plane]
                    c2 = cb2.ap()[:, plane]
                    barrier()
                    emit(vector.tensor_scalar_mul(c1, vv(0), mcol(s, 0)))
                    barrier()
                    emit(
                        vector.scalar_tensor_tensor(
                            out=c2, in0=vv(1), scalar=mcol(s, 1), in1=c1,
                            op0=OP.mult, op1=OP.add,
                        )
                    )
                    barrier()
                    emit(
                        vector.scalar_tensor_tensor(
                            out=c1, in0=vv(2), scalar=mcol(s, 2), in1=c2,
                            op0=OP.mult, op1=OP.add,
                        )
                    )
                    if s >= N_OBUFS and plane == 0:
                        vector.wait_ge(dmaouts[s - N_OBUFS], 16)
                    barrier()
                    emit(
                        vector.scalar_tensor_tensor(
                            out=obufs[s % N_OBUFS].ap()[:, plane], in0=vv(3),
                            scalar=mcol(s, 3), in1=c1, op0=OP.mult, op1=OP.add,
                        )
                    )

            def ch(s):
                if s == LAST:
                    ch_planes(s)
                    return
                barrier()
                emit(vector.tensor_scalar_mul(sq_view(cb1), V(s, 0), mcol(s, 0)))
                barrier()
                emit(
                    vector.scalar_tensor_tensor(
                        out=cb2.ap(), in0=V(s, 1), scalar=mcol(s, 1), in1=cb1.ap(),
                        op0=OP.mult, op1=OP.add,
                    )
                )
                barrier()
                emit(
                    vector.scalar_tensor_tensor(
                        out=cb1.ap(), in0=V(s, 2), scalar=mcol(s, 2), in1=cb2.ap(),
                        op0=OP.mult, op1=OP.add,
                    )
                )
                if s >= N_OBUFS:
                    vector.wait_ge(dmaouts[s - N_OBUFS], 16)
                barrier()
                emit(
                    vector.scalar_tensor_tensor(
                        out=obufs[s % N_OBUFS].ap(), in0=V(s, 3), scalar=mcol(s, 3),
                        in1=cb1.ap(), op0=OP.mult, op1=OP.add,
                    )
                )

            fns = {"mk": mk, "ch": ch, "vn": vn}
            for g, s in VE_ORDER:
                fns[g](s)
            assert cnt[0] == VE_TOTAL

    return nc


_NC_CACHE = None


def _get_nc():
    global _NC_CACHE
    if _NC_CACHE is None:
        _NC_CACHE = build_nc()
    return _NC_CACHE


def kernel(x) -> np.ndarray:
    x = np.asarray(x, dtype=np.float32)
    assert x.shape == (B, C, H, W), x.shape
    shards = np.split(x, NCORES, axis=0)
    in_maps = [{"x": s} for s in shards]
    res = run_bass_kernel_spmd(_get_nc(), in_maps, core_ids=list(range(NCORES)))
    return np.concatenate([r["out"] for r in res.results], axis=0)
